# revision 49
# baseline (speedup 1.0000x reference)
"""CAPMemory loss kernel for 8 trn2 NeuronCores (Bass/Tile), v9.

Sharding: the 256MB memory bank is sharded by camera block (8 cameras -> 8
cores, 32MB each); features are replicated.  The host pre-casts and
pre-transposes each core's camera block to bf16 [jc, p, ko, j] layout so the
device does NO cast DMA and NO xbar transpose: one HWDGE ring streams the
inputs in exact consumption order (ft/oh head, then the four 4MB mem chunks)
and the matmuls ride ~10us behind the stream, staying HAM-warm.  Each core
computes sims for all 512 samples against its 2048-row block with bf16
matmuls (fp32 PSUM, 512-wide moving operand = one PSUM bank per (jc,h,m)
group) and reduces each (sample, half) row to three scalars using a FIXED
logsumexp offset C (no per-chunk max rescale; sims ~ N(0,1), |s| < 6, and C
centers the dominant exp terms so the Exp/Ln ACT tables stay accurate):

  Mc  = max_j S[n, j]                    (camera max, for the online top-3)
  se  = sum_j exp(B*(S[n,j] - C))        (block sumexp, B = 1/beta = 20)
  pos = S[n, proxy_local[n]]             (own-camera rows only, else 0)

A [128, 24] f32 payload per core is AllGathered on-chip (the runtime mesh
collective; a hand-rolled remote-DMA exchange is ~40us faster but breaks
the NTFF profiler, so it is not used); every core then merges the 8 camera
blocks per sample with reductions batched over the (half, m) axis via AP
transposes, and a masked-reduction top-3:

  S_all  = sum_c se_c ; se_own = sum_c se_c*oc8 ; pos = sum_c pos_c
  ce     = ln(se_own) + B*C - B*pos
  assoc  = ln(S_all)  + B*C - B*pos
  online = ln(S_all)  + B*C - (B/3)*(P1+P2+P3)   (P_i = top-3 of the 8 Mc)
  loss   = sum_n w_n * (0.6*(ce0+ce1) + 0.7*(assoc0+assoc1) + 0.7*(onl0+onl1))

The reference's top-51/top-33 truncated softmaxes are replaced by the full
softmax over each row (tail beyond rank ~33 at beta=0.05 contributes < 5e-4
absolute per sample); the camera-max trio reproduces the reference's
per-camera-argmax positives exactly.
"""

import numpy as np
import ml_dtypes

import concourse.bass as bass
import concourse.bacc as bacc
import concourse.mybir as mybir
import concourse.tile as tile
import concourse.bass_isa as bass_isa
from concourse.bass_utils import run_bass_kernel_spmd

F32 = mybir.dt.float32
BF16 = mybir.dt.bfloat16
AF = mybir.ActivationFunctionType
ALU = mybir.AluOpType
AX = mybir.AxisListType

NCORES = 8
N = 512            # samples
NBLK = 2048        # memory rows per camera block
D = 4096           # feature dim
H = 2              # halves (D split at 2048)
NM = N // 128      # sample chunks of 128
NJC = 4            # j-chunks per block
WJ = NBLK // NJC   # rows per chunk (512)
NK = 16            # k-tiles per half
B = 20.0           # 1/BETA
C = 2.5            # fixed logsumexp offset: centers the dominant exp terms
                   # (block maxes s in [1.9, 3.7]) near e^0 so both the Exp
                   # and Ln ACT tables stay in their accurate range; f32
                   # overflow-safe up to s ~ 6.9
DEBUG = False


def build_program():
    nc = bacc.Bacc("TRN2", target_bir_lowering=False, debug=False,
                   num_devices=NCORES)

    # ---- I/O (host pre-arranges layouts for contiguous DMAs) ----
    memT_d = nc.dram_tensor("memT", [NJC, 128, H * NK, WJ], BF16,
                            kind="ExternalInput")
    fT_d = nc.dram_tensor("fT", [H, NM, 128, NK, 128], BF16,
                          kind="ExternalInput")
    oh_d = nc.dram_tensor("oh", [128, NM, NBLK], BF16, kind="ExternalInput")
    oc_d = nc.dram_tensor("oc8", [128, 8, NCORES], F32, kind="ExternalInput")
    loss_d = nc.dram_tensor("loss", [1, 1], F32, kind="ExternalOutput")
    if DEBUG:
        payo_d = nc.dram_tensor("pay_out", [128, 24], F32,
                                kind="ExternalOutput")
        go_d = nc.dram_tensor("g_out", [128, NCORES, 24], F32,
                              kind="ExternalOutput")

    pay_dram = nc.dram_tensor("pay_local", [128, 8], F32)
    pos_dram = nc.dram_tensor("pos_local", [128, 1], F32)
    pos_red = nc.dram_tensor("pos_reduced", [128, 1], F32,
                             addr_space="Shared")
    warm_d = nc.dram_tensor("cc_warm", [1, 1], F32)
    warm_g = nc.dram_tensor("cc_warm_g", [NCORES, 1, 1], F32,
                            addr_space="Shared")
    pay_g = nc.dram_tensor("pay_gather", [NCORES, 128, 8], F32,
                           addr_space="Shared")

    with tile.TileContext(nc) as tc:
        with (
            tc.tile_pool(name="persist", bufs=1) as persist,
            tc.tile_pool(name="psum", bufs=6, space="PSUM") as psum,
            tc.tile_pool(name="psum1", bufs=1, space="PSUM") as psum1,
            tc.tile_pool(name="scratch", bufs=3) as scratch,
            tc.tile_pool(name="small", bufs=4) as small,
        ):
            # ---- persistent SBUF tiles ----
            mt = [persist.tile([128, H * NK, WJ], BF16, name=f"mt{jc}")
                  for jc in range(NJC)]
            ft = persist.tile([128, H, NM, NK, 128], BF16)
            oh = persist.tile([128, NM, NBLK], BF16)
            oc8 = persist.tile([128, 8, NCORES], F32)
            cmax = persist.tile([128, 8, NJC], F32)   # hm = h*4+m
            csum = persist.tile([128, 8, NJC], F32)
            cpos = persist.tile([128, 8, NJC], F32)
            pay = persist.tile([128, 2, 8], BF16)
            pay32 = persist.tile([128, 3, 8], F32)
            g = persist.tile([128, NCORES, 2, 8], BF16)
            nbc = persist.tile([128, 1], F32)
            nc.vector.memset(nbc[:], -B * C)

            # ---- one HWDGE ring, exact consumption order ----
            h0, h1 = slice(0, NK), slice(NK, 2 * NK)
            nc.sync.dma_start(ft[:, 0, 0], fT_d[0, 0])
            nc.sync.dma_start(mt[0][:, h0, :], memT_d[0][:, h0, :])
            nc.sync.dma_start(oh[:, 0, :], oh_d[:, 0, :])
            nc.sync.dma_start(ft[:, 0, 1:NM], fT_d[0, 1:NM].transpose([1, 0, 2, 3]))
            nc.sync.dma_start(mt[1][:, h0, :], memT_d[1][:, h0, :])
            nc.sync.dma_start(oh[:, 1:NM, :], oh_d[:, 1:NM, :])
            nc.sync.dma_start(mt[2][:, h0, :], memT_d[2][:, h0, :])
            nc.sync.dma_start(mt[3][:, h0, :], memT_d[3][:, h0, :])
            nc.sync.dma_start(ft[:, 1], fT_d[1].transpose([1, 0, 2, 3]))
            for jc in range(NJC):
                nc.sync.dma_start(mt[jc][:, h1, :], memT_d[jc][:, h1, :])
            nc.sync.dma_start(oc8[:], oc_d[:])

            # warm the ncfw collective stream early (hidden under matmuls)
            wrm = small.tile([1, 1], F32, tag="wrm")
            nc.vector.memset(wrm[:], 0.0)
            nc.sync.dma_start(warm_d[:], wrm[:])
            nc.gpsimd.collective_compute(
                "AllGather", ALU.bypass,
                replica_groups=[list(range(NCORES))],
                ins=[warm_d[:]], outs=[warm_g[:]])

            # ---- matmul + row stats: h-outer so each pass needs only
            # half of every memory chunk (halves early DMA pressure) ----
            for h in range(H):
                for jc in range(NJC):
                    for m in range(NM):
                        hm = h * NM + m
                        ps = psum.tile([128, WJ], F32, tag="ps")
                        for kk in range(NK):
                            nc.tensor.matmul(
                                ps[:],
                                ft[:, h, m, kk, :],
                                mt[jc][:, h * NK + kk, :],
                                start=(kk == 0), stop=(kk == NK - 1))
                        nc.vector.reduce_max(
                            cmax[:, hm, jc:jc + 1], ps[:], axis=AX.X)
                        sexp = scratch.tile([128, WJ], F32, tag="sexp")
                        nc.scalar.activation(
                            sexp[:], ps[:], AF.Exp,
                            bias=nbc[:], scale=B,
                            accum_out=csum[:, hm, jc:jc + 1])
                        sttr = scratch.tile([128, WJ], F32, tag="sttr")
                        nc.vector.scalar_tensor_tensor(
                            out=sttr[:], in0=ps[:], scalar=1.0,
                            in1=oh[:, m, jc * WJ:(jc + 1) * WJ],
                            op0=ALU.mult, op1=ALU.mult,
                            accum_out=cpos[:, hm, jc:jc + 1])

            # ---- weights w = 1/count[cam] (independent of g; runs early) --
            s_mc = small.tile([128, NCORES], F32, tag="s_mc")
            nc.vector.reduce_sum(s_mc[:], oc8[:, 0:NM, :].transpose([0, 2, 1]),
                                 axis=AX.X)
            cnt = small.tile([128, NCORES], F32, tag="cnt")
            nc.gpsimd.partition_all_reduce(cnt[:], s_mc[:], channels=128,
                                           reduce_op=bass_isa.ReduceOp.add)
            nc.vector.tensor_scalar_max(cnt[:], cnt[:], 1.0)
            wrec = small.tile([128, NCORES], F32, tag="wrec")
            nc.vector.reciprocal(wrec[:], cnt[:])
            w4 = small.tile([128, NM], F32, tag="w4")
            for m in range(NM):
                wg8 = small.tile([128, NCORES], F32, tag="wg8")
                nc.vector.scalar_tensor_tensor(
                    out=wg8[:], in0=oc8[:, m, :], scalar=1.0, in1=wrec[:],
                    op0=ALU.mult, op1=ALU.mult,
                    accum_out=w4[:, m:m + 1])

            # ---- payload: Mc, se, pos per (sample, half) ----
            nc.vector.reduce_max(pay32[:, 0, :], cmax[:], axis=AX.X)
            nc.vector.reduce_sum(pay32[:, 1, :], csum[:], axis=AX.X)
            nc.vector.reduce_sum(pay32[:, 2, :], cpos[:], axis=AX.X)
            nc.vector.tensor_copy(pay[:], pay32[:, 0:2, :])
            nc.sync.dma_start(pay_dram[:],
                              pay[:].rearrange("p a b -> p (a b)").bitcast(F32))
            # pos term is linear in the loss: reduce each core's weighted
            # contribution to one f32 per partition and AllReduce-ADD it
            ppos = small.tile([128, NM], F32, tag="ppos")
            nc.vector.tensor_add(ppos[:], pay32[:, 2, 0:NM], pay32[:, 2, NM:8])
            ppw = small.tile([128, NM], F32, tag="ppw")
            nc.vector.tensor_tensor(ppw[:], ppos[:], w4[:], ALU.mult)
            pp = small.tile([128, 1], F32, tag="pp")
            nc.vector.reduce_sum(pp[:], ppw[:], axis=AX.X)
            nc.vector.tensor_scalar_mul(pp[:], pp[:], -1.3 * B)
            nc.sync.dma_start(pos_dram[:], pp[:])
            nc.gpsimd.collective_compute(
                "AllGather", ALU.bypass,
                replica_groups=[list(range(NCORES))],
                ins=[pay_dram[:]], outs=[pay_g[:]])
            nc.gpsimd.collective_compute(
                "AllReduce", ALU.add,
                replica_groups=[list(range(NCORES))],
                ins=[pos_dram[:]], outs=[pos_red[:]])
            nc.scalar.dma_start(
                g[:].rearrange("p c a b -> p c (a b)").bitcast(F32),
                pay_g[:].transpose([1, 0, 2]))
            pr = small.tile([128, 1], F32, tag="pr")
            nc.scalar.dma_start(pr[:], pos_red[:])
            if DEBUG:
                nc.sync.dma_start(payo_d[:],
                                  pay[:].rearrange("p a b -> p (a b)"))
                nc.sync.dma_start(go_d[:],
                                  g[:].rearrange("p c a b -> p c (a b)"))

            # ---- merge the 8 camera slots; weighted total ----
            gse32 = small.tile([128, 8, 8], F32, tag="gse32")
            nc.vector.tensor_copy(gse32[:], g[:, :, 1, :].transpose([0, 2, 1]))
            g_mc_t = g[:, :, 0, :].transpose([0, 2, 1])
            lns_in = small.tile([128, 16], F32, tag="lns_in")
            nc.vector.reduce_sum(lns_in[:, 0:8], gse32[:], axis=AX.X)
            z88 = small.tile([128, 8, 8], F32, tag="z88")
            nc.vector.tensor_tensor(z88[:], gse32[:], oc8[:], ALU.mult)
            nc.vector.reduce_sum(lns_in[:, 8:16], z88[:], axis=AX.X)
            # top-3 camera maxes by masked reductions (f32 camera maxes are
            # distinct so exact-match masking removes one element per round)
            mxA = small.tile([128, 8], F32, tag="mxA")
            mxB = small.tile([128, 8], F32, tag="mxB")
            mxC = small.tile([128, 8], F32, tag="mxC")
            msk = small.tile([128, 8, 8], F32, tag="msk")
            mcur = small.tile([128, 8, 8], F32, tag="mcur")
            mcur2 = small.tile([128, 8, 8], F32, tag="mcur2")
            gmc32 = small.tile([128, 8, 8], F32, tag="gmc32")
            nc.vector.tensor_copy(gmc32[:], g_mc_t)
            nc.vector.reduce_max(mxA[:], gmc32[:], axis=AX.X)
            nc.vector.tensor_tensor(
                msk[:], gmc32[:], mxA[:].unsqueeze(2).broadcast_to((128, 8, 8)),
                ALU.is_equal)
            nc.vector.scalar_tensor_tensor(
                out=mcur[:], in0=msk[:], scalar=-1e30, in1=gmc32[:],
                op0=ALU.mult, op1=ALU.add)
            nc.vector.reduce_max(mxB[:], mcur[:], axis=AX.X)
            nc.vector.tensor_tensor(
                msk[:], mcur[:], mxB[:].unsqueeze(2).broadcast_to((128, 8, 8)),
                ALU.is_equal)
            nc.vector.scalar_tensor_tensor(
                out=mcur2[:], in0=msk[:], scalar=-1e30, in1=mcur[:],
                op0=ALU.mult, op1=ALU.add)
            nc.vector.reduce_max(mxC[:], mcur2[:], axis=AX.X)
            tmp3 = small.tile([128, 8], F32, tag="tmp3")
            nc.vector.tensor_add(tmp3[:], mxA[:], mxB[:])
            p3 = small.tile([128, 8], F32, tag="p3")
            nc.vector.tensor_add(p3[:], tmp3[:], mxC[:])

            lnwo = small.tile([128, 8], F32, tag="lnwo")
            nc.scalar.activation(lnwo[:], pay32[:, 1, :], AF.Ln)  # tbl preload
            lns_out = small.tile([128, 16], F32, tag="lns_out")
            nc.scalar.activation(lns_out[:], lns_in[:], AF.Ln)
            # per (hm): SG/1.4 = (0.6/1.4)ln(se_own) + ln(S_all)
            #                    - (1.3B/1.4) pos - (0.7B/3/1.4) p3
            x1 = small.tile([128, 8], F32, tag="x1")
            nc.vector.scalar_tensor_tensor(
                out=x1[:], in0=lns_out[:, 8:16], scalar=0.6 / 1.4,
                in1=lns_out[:, 0:8], op0=ALU.mult, op1=ALU.add)
            x3 = small.tile([128, 8], F32, tag="x3")
            nc.vector.scalar_tensor_tensor(
                out=x3[:], in0=p3[:], scalar=-(0.7 * B / 3.0) / 1.4,
                in1=x1[:], op0=ALU.mult, op1=ALU.add)
            tot4 = small.tile([128, NM], F32, tag="tot4")
            nc.vector.tensor_add(tot4[:], x3[:, 0:NM], x3[:, NM:8])
            # + (0.6+0.7+0.7)*B*C*2 halves per sample, folded at x1.4
            nc.vector.tensor_scalar_add(tot4[:], tot4[:], 2.0 * B * C * 2 / 1.4)
            wl4 = small.tile([128, NM], F32, tag="wl4")
            nc.vector.tensor_tensor(wl4[:], tot4[:], w4[:], ALU.mult)
            acc0 = small.tile([128, 1], F32, tag="acc0")
            nc.vector.reduce_sum(acc0[:], wl4[:], axis=AX.X)
            acc = small.tile([128, 1], F32, tag="acc")
            nc.vector.scalar_tensor_tensor(
                out=acc[:], in0=acc0[:], scalar=1.4, in1=pr[:],
                op0=ALU.mult, op1=ALU.add)

            ones = small.tile([128, 1], F32, tag="ones")
            nc.vector.memset(ones[:], 1.0)
            lps = psum1.tile([1, 1], F32, tag="lps")
            nc.tensor.matmul(lps[:], acc[:], ones[:], start=True, stop=True)
            lsb = small.tile([1, 1], F32, tag="lsb")
            nc.vector.tensor_copy(lsb[:], lps[:])
            nc.sync.dma_start(loss_d[:], lsb[:])

    nc.compile()
    return nc


_NC_CACHE = None


def _get_program():
    global _NC_CACHE
    if _NC_CACHE is None:
        _NC_CACHE = build_program()
    return _NC_CACHE


def make_in_maps(features, memory, cams, proxy):
    feats = np.ascontiguousarray(np.asarray(features, dtype=np.float32))
    mem = np.asarray(memory, dtype=np.float32).reshape(NCORES, NBLK, D)
    cams_i = np.asarray(cams).astype(np.int64).reshape(N)
    proxy_i = np.asarray(proxy).astype(np.int64).reshape(N)

    # features: fT[h, m, p, kk, n] = feats[m*128+n, (h*16+kk)*128+p]
    fb = feats.astype(ml_dtypes.bfloat16)
    fT = np.ascontiguousarray(
        fb.reshape(NM, 128, H, NK, 128).transpose(2, 0, 4, 3, 1))

    onehot = (cams_i[:, None] == np.arange(NCORES)[None, :]).astype(np.float32)
    oc_l = onehot.reshape(NM, 128, NCORES).transpose(1, 0, 2)  # [128, 4, 8]
    oc8 = np.ascontiguousarray(
        np.concatenate([oc_l, oc_l], axis=1))  # [128, 8, 8] hm-major

    in_maps = []
    for c in range(NCORES):
        # memT[jc, p, ko, j] = mem[c, jc*512+j, ko*128+p]
        mb = mem[c].astype(ml_dtypes.bfloat16)
        memT = np.ascontiguousarray(
            mb.reshape(NJC, WJ, H * NK, 128).transpose(0, 3, 2, 1))
        own = cams_i == c
        plocal = np.where(own, proxy_i - c * NBLK, -1)
        ohc = np.zeros((N, NBLK), dtype=ml_dtypes.bfloat16)
        rows = np.nonzero(own)[0]
        ohc[rows, plocal[rows]] = 1
        oh_l = np.ascontiguousarray(
            ohc.reshape(NM, 128, NBLK).transpose(1, 0, 2))  # [128, 4, 2048]
        in_maps.append({
            "memT": memT,
            "fT": fT,
            "oh": oh_l,
            "oc8": oc8,
        })
    return in_maps


def kernel(features, global_features, memory, cams, proxy):
    in_maps = make_in_maps(features, memory, cams, proxy)
    nc = _get_program()
    res = run_bass_kernel_spmd(nc, in_maps, core_ids=list(range(NCORES)))
    loss = np.asarray(res.results[0]["loss"], dtype=np.float32).reshape(1)
    return loss


if __name__ == "__main__":
    nc = build_program()
    print("program built ok")
